# revision 11
# baseline (speedup 1.0000x reference)
"""Trainium2 Bass kernel for ClassicalSelfAttention.

  out = softmax((x @ Wq) @ (x @ Wk)^T / sqrt(D)) @ x      x: [8192, 1024] f32

Sharding (8 NeuronCores): rows of x are sharded across cores; each core
projects its own row-shard to Q^T and K^T, the K^T shards are AllGathered
across cores (SDMA, overlaps compute), and each core runs a streaming
attention loop over 16 key-blocks of 512 keys: scores matmul -> fused
exp+rowsum on ScalarE -> PE transpose of the prob block -> PV matmul
accumulated in SBUF. The softmax division is folded into the final output
scale. 1/sqrt(1024) = 2^-5 is folded into Wq on the host (exact in fp32).

Projections run in float32r (full PE rate, near-fp32 accuracy); the
scores and PV matmuls run in bf16 with fp32 PSUM accumulation. The scores
matmul keeps K^T stationary and Q^T moving, so PSUM holds scores
TRANSPOSED ([key, query]); exp of that is P^T directly -- which is
exactly the layout the PV matmul needs as its stationary operand -- so no
PE transposes are required at all. The softmax row-sums (a partition-dim
reduction in this layout) are computed by a ones-vector matmul and fixed
up into per-partition scalars at the end via a DRAM bounce.
To hide the AllGather latency each core processes its OWN
key blocks first straight out of SBUF (plus its own V rows from a
per-core x_shard input); the remaining 14 key blocks are fetched in
rank-rotated order (rank + j) % 8 via partition-id-based dynamic DMA
offsets, so no core waits on the gather before doing useful work.
Softmax over key blocks is order-invariant, so the rotation is free.
"""

import sys

import numpy as np

try:
    import concourse.bass as bass  # noqa: F401
except ImportError:  # pragma: no cover
    sys.path.insert(0, "/opt/trn_rl_repo")

import concourse.bacc as bacc
import concourse.mybir as mybir
import concourse.tile as tile
from concourse.masks import make_identity
from concourse import bass_utils
from concourse.bass import ds

N_TOKENS = 8192
EMBED = 1024
NCORES = 8
M = N_TOKENS // NCORES  # rows per core (1024)
P = 128  # partitions
DC = EMBED // P  # contraction chunks (8)
NB = 512  # key-block width
NNB = N_TOKENS // NB  # key blocks (16)
MB = M // P  # query row-blocks per core (8)
VC = NB // P  # value chunks per key block (4)
HPR = M // NB  # key-block halves per rank (2)
FP32 = mybir.dt.float32
R32 = mybir.dt.float32r
BF16 = mybir.dt.bfloat16
EXP = mybir.ActivationFunctionType.Exp
ADD = mybir.AluOpType.add
AXX = mybir.AxisListType.X


def _build():
    nc = bacc.Bacc(
        "TRN2", target_bir_lowering=False, debug=False, num_devices=NCORES
    )
    xt_shard = nc.dram_tensor("xt_shard", [EMBED, M], R32, kind="ExternalInput").ap()
    x_shard = nc.dram_tensor("x_shard", [M, EMBED], BF16, kind="ExternalInput").ap()
    x_full = nc.dram_tensor(
        "x_full", [N_TOKENS, EMBED], BF16, kind="ExternalInput"
    ).ap()
    wq_d = nc.dram_tensor("wq", [EMBED, EMBED], R32, kind="ExternalInput").ap()
    wk_d = nc.dram_tensor("wk", [EMBED, EMBED], R32, kind="ExternalInput").ap()
    out_d = nc.dram_tensor("out", [M, EMBED], FP32, kind="ExternalOutput").ap()

    wq_r = wq_d.rearrange("(a p) d -> a p d", p=P)  # [DC, P, EMBED]
    wk_r = wk_d.rearrange("(a p) d -> a p d", p=P)
    xt_r = xt_shard.rearrange("(a p) m -> a p m", p=P)  # [DC, P, M]
    xs_r = x_shard.rearrange("(t p) d -> t p d", p=P)  # [M//P, P, EMBED]
    xv_r = x_full.rearrange("(t p) d -> t p d", p=P)  # [64, P, EMBED]
    out_r = out_d.rearrange("(t p) d -> t p d", p=P)  # [MB, P, EMBED]

    with tile.TileContext(nc) as tc:
        with (
            tc.tile_pool(name="persist", bufs=1) as pers,
            tc.tile_pool(name="persist_dram", bufs=1, space="DRAM") as pdram,
        ):
            ones_sb = pers.tile([P, P], BF16)
            nc.vector.memset(ones_sb[:], 1.0)
            ident = pers.tile([P, P], FP32)
            make_identity(nc, ident[:])
            # Q^T resident for the whole kernel: qt[p, b*M + m] = Qt[b*128+p, m]
            qt = pers.tile([P, DC * M], BF16)
            # own K^T shard, kept resident: ktsb[p, b*M + n] = Kt_own[b*128+p, n]
            ktsb = pers.tile([P, DC * M], BF16)
            # fp32 PV accumulator per query block: [p, mb*EMBED + dv]
            out_acc = pers.tile([P, MB * EMBED], FP32)
            # softmax denominators, replicated across partitions: [p, m]
            sums_acc = pers.tile([P, M], FP32)
            # K^T shard (AllGather input) and gathered K^T of all cores
            ktd = pdram.tile([DC, P, M], BF16)
            gkt = pdram.tile([NCORES * DC, P, M], BF16, addr_space="Shared")

            rank = nc.gpsimd.partition_id()

            # ---- Phase A: project Q^T (own rows) and K^T shard, AllGather K^T
            with (
                tc.tile_pool(name="proj", bufs=1) as proj,
                tc.tile_pool(name="proj_ps", bufs=4, space="PSUM") as proj_ps,
            ):
                wq_sb = proj.tile([P, DC * EMBED], R32)
                wk_sb = proj.tile([P, DC * EMBED], R32)
                xt_sb = proj.tile([P, DC * M], R32)
                for a in range(DC):
                    nc.sync.dma_start(
                        out=wk_sb[:, a * EMBED : (a + 1) * EMBED], in_=wk_r[a]
                    )
                    nc.sync.dma_start(
                        out=xt_sb[:, a * M : (a + 1) * M], in_=xt_r[a]
                    )
                    nc.sync.dma_start(
                        out=wq_sb[:, a * EMBED : (a + 1) * EMBED], in_=wq_r[a]
                    )
                # K^T first so its AllGather overlaps the Q^T projection.
                for w_sb, dst in ((wk_sb, ktsb), (wq_sb, qt)):
                    for b in range(DC):  # output dim chunk
                        for j in range(M // NB):  # row half
                            ps = proj_ps.tile([P, NB], FP32, tag="proj_ps")
                            for a in range(DC):  # contraction chunk
                                nc.tensor.matmul(
                                    ps[:],
                                    lhsT=w_sb[:, a * EMBED + b * P : a * EMBED + (b + 1) * P],
                                    rhs=xt_sb[:, a * M + j * NB : a * M + (j + 1) * NB],
                                    start=(a == 0),
                                    stop=(a == DC - 1),
                                )
                            nc.vector.tensor_copy(
                                out=dst[:, b * M + j * NB : b * M + (j + 1) * NB],
                                in_=ps[:],
                            )
                    if dst is ktsb:
                        for b in range(DC):
                            nc.sync.dma_start(
                                out=ktd[b], in_=ktsb[:, b * M : (b + 1) * M]
                            )
                        nc.gpsimd.collective_compute(
                            "AllGather",
                            mybir.AluOpType.bypass,
                            replica_groups=[list(range(NCORES))],
                            ins=[ktd.opt()],
                            outs=[gkt.opt()],
                        )

            # ---- Phase B: streaming attention over key blocks, own rank first
            with (
                tc.tile_pool(name="kv", bufs=2) as kvp,
                tc.tile_pool(name="pb", bufs=3) as pbp,
                tc.tile_pool(name="ps_s", bufs=3, space="PSUM") as ps_sp,
                tc.tile_pool(name="ps_u", bufs=2, space="PSUM") as ps_up,
                tc.tile_pool(name="ps_o", bufs=2, space="PSUM") as ps_op,
            ):
                for nb in range(NNB):  # local processing order
                    j, half = nb // HPR, nb % HPR  # j = rank offset
                    vtile = kvp.tile([P, VC * EMBED], BF16, tag="vtile")
                    if j == 0:
                        # own keys: K^T already in SBUF, V rows from x_shard
                        for c in range(VC):
                            nc.sync.dma_start(
                                out=vtile[:, c * EMBED : (c + 1) * EMBED],
                                in_=xs_r[half * VC + c],
                            )
                        k_sb, k_off = ktsb, half * NB

                        def k_slice(b):
                            return ktsb[:, b * M + k_off : b * M + k_off + NB]

                    else:
                        src = (rank + j) % NCORES
                        for c in range(VC):
                            nc.gpsimd.dma_start(
                                out=vtile[:, c * EMBED : (c + 1) * EMBED],
                                in_=xv_r[
                                    ds(src * (M // P) + half * VC + c, 1)
                                ].squeeze(0),
                            )
                        ktile = kvp.tile([P, DC * NB], BF16, tag="ktile")
                        for b in range(DC):
                            nc.gpsimd.dma_start(
                                out=ktile[:, b * NB : (b + 1) * NB],
                                in_=gkt[
                                    ds(src * DC + b, 1),
                                    :,
                                    half * NB : (half + 1) * NB,
                                ].squeeze(0),
                            )

                        def k_slice(b, _kt=ktile):
                            return _kt[:, b * NB : (b + 1) * NB]

                    pt_sb = pbp.tile([P, VC * M], BF16, tag="pt_sb")
                    for h in range(M // NB):  # query column half
                        for c in range(VC):  # key chunk within block
                            ps_s = ps_sp.tile([P, NB], FP32, tag="ps_s")
                            for b in range(DC):
                                nc.tensor.matmul(
                                    ps_s[:],
                                    lhsT=k_slice(b)[:, c * P : (c + 1) * P],
                                    rhs=qt[:, b * M + h * NB : b * M + (h + 1) * NB],
                                    start=(b == 0),
                                    stop=(b == DC - 1),
                                )
                            nc.scalar.activation(
                                out=pt_sb[:, c * M + h * NB : c * M + (h + 1) * NB],
                                in_=ps_s[:],
                                func=EXP,
                            )
                    # partition-dim softmax sums via ones-vector matmul
                    for h in range(M // NB):
                        ps_sum = ps_up.tile([P, NB], FP32, tag="ps_sum")
                        for c in range(VC):
                            nc.tensor.matmul(
                                ps_sum[:],
                                lhsT=ones_sb[:],
                                rhs=pt_sb[:, c * M + h * NB : c * M + (h + 1) * NB],
                                start=(c == 0),
                                stop=(c == VC - 1),
                            )
                        dsts = sums_acc[:, h * NB : (h + 1) * NB]
                        if nb == 0:
                            nc.vector.tensor_copy(out=dsts, in_=ps_sum[:])
                        else:
                            nc.vector.tensor_tensor(
                                out=dsts, in0=dsts, in1=ps_sum[:], op=ADD
                            )
                    for mb in range(MB):
                        for h in range(EMBED // NB):
                            ps_o = ps_op.tile([P, NB], FP32, tag="ps_o")
                            for t in range(VC):
                                nc.tensor.matmul(
                                    ps_o[:],
                                    lhsT=pt_sb[:, t * M + mb * P : t * M + (mb + 1) * P],
                                    rhs=vtile[:, t * EMBED + h * NB : t * EMBED + (h + 1) * NB],
                                    start=(t == 0),
                                    stop=(t == VC - 1),
                                )
                            dst = out_acc[:, mb * EMBED + h * NB : mb * EMBED + (h + 1) * NB]
                            if nb == 0:
                                nc.vector.tensor_copy(out=dst, in_=ps_o[:])
                            else:
                                nc.vector.tensor_tensor(
                                    out=dst, in0=dst, in1=ps_o[:], op=ADD
                                )

            # ---- Phase C: divide by softmax sum, write out
            with (
                tc.tile_pool(name="fin", bufs=2) as fin,
                tc.tile_pool(name="fin_ps", bufs=2, space="PSUM") as fin_ps,
            ):
                scol = fin.tile([P, MB], FP32)
                for mb in range(MB):
                    ps_f = fin_ps.tile([P, P], FP32, tag="ps_f")
                    nc.tensor.transpose(
                        out=ps_f[:],
                        in_=sums_acc[:, mb * P : (mb + 1) * P],
                        identity=ident[:],
                    )
                    nc.vector.tensor_copy(
                        out=scol[:, mb : mb + 1], in_=ps_f[:, 0:1]
                    )
                rtot = fin.tile([P, MB], FP32)
                nc.vector.reciprocal(out=rtot[:], in_=scol[:])
                for mb in range(MB):
                    outf = fin.tile([P, EMBED], FP32, tag="outf")
                    nc.vector.tensor_scalar_mul(
                        outf[:],
                        out_acc[:, mb * EMBED : (mb + 1) * EMBED],
                        rtot[:, mb : mb + 1],
                    )
                    nc.sync.dma_start(out=out_r[mb], in_=outf[:])

    nc.compile()
    return nc


_NC = None


def _get_nc():
    global _NC
    if _NC is None:
        _NC = _build()
    return _NC


def _run(x, rotation_params, entangle_params, **spmd_kwargs):
    x = np.ascontiguousarray(np.asarray(x, dtype=np.float32))
    wq = np.asarray(rotation_params, dtype=np.float32).reshape(EMBED, EMBED) * np.float32(
        1.0 / 32.0
    )
    wk = np.asarray(entangle_params, dtype=np.float32).reshape(EMBED, EMBED)
    xt = np.ascontiguousarray(x.T)
    import ml_dtypes

    x_bf = x.astype(ml_dtypes.bfloat16)
    in_maps = [
        {
            "xt_shard": np.ascontiguousarray(xt[:, i * M : (i + 1) * M]),
            "x_shard": np.ascontiguousarray(x_bf[i * M : (i + 1) * M]),
            "x_full": x_bf,
            "wq": wq,
            "wk": wk,
        }
        for i in range(NCORES)
    ]
    res = bass_utils.run_bass_kernel_spmd(
        _get_nc(), in_maps, core_ids=list(range(NCORES)), **spmd_kwargs
    )
    out = np.concatenate([res.results[i]["out"] for i in range(NCORES)], axis=0)
    return out, res


def kernel(x, rotation_params, entangle_params):
    out, _ = _run(x, rotation_params, entangle_params)
    return out
